# revision 15
# baseline (speedup 1.0000x reference)
import numpy as np
import ml_dtypes
import jax
import jax.numpy as jnp
from jax.sharding import Mesh, PartitionSpec as P, NamedSharding
from jax.experimental.shard_map import shard_map

# nn_GatedFusionBlockCustom: B=8, S=2048, H=256, NH=8 heads.
# Data-parallel over batch: one batch element per NeuronCore (8 cores).
# The host<->device link is the bottleneck (~40 MB/s, plus ~60-90ms per
# transfer RPC), so:
#   - video/audio are quantized to int8 on host (per-tensor scale,
#     dequantized on device),
#   - matmul weights are packed bf16, sharded across the 8 cores on the wire
#     and all-gathered on device over NeuronLink,
#   - the global gating MLPs run on host in f32 (they only need means over S),
#     and their broadcast outputs are materialized host-side (saves 32MB d2h),
#   - `final` comes back int8 with a per-core scale,
#   - ALL per-core inputs ride in ONE packed uint8 buffer (one h2d RPC) and
#     the output is ONE packed uint8 buffer (one d2h RPC).
B, S, H, NH = 8, 2048, 256, 8
DH = H // NH
BF16 = ml_dtypes.bfloat16

MM_KEYS = ['aproj_w', 'outproj_w', 'ffn1_w1', 'ffn1_w2', 'ffn2_w1', 'ffn2_w2',
           'attn_in_w', 'attn_out_w']
SMALL_KEYS = ['aproj_b', 'outproj_b', 'ffn1_b1', 'ffn1_b2', 'ffn2_b1', 'ffn2_b2',
              'attn_in_b', 'attn_out_b', 'n1_g', 'n1_b', 'n2_g', 'n2_b',
              'n3_g', 'n3_b', 'n4_g', 'n4_b']

_MM_SHAPES = {'aproj_w': (H, H), 'outproj_w': (H, H),
              'ffn1_w1': (4 * H, H), 'ffn1_w2': (H, 4 * H),
              'ffn2_w1': (4 * H, H), 'ffn2_w2': (H, 4 * H),
              'attn_in_w': (3 * H, H), 'attn_out_w': (H, H)}
_MM_SIZES = [int(np.prod(_MM_SHAPES[k])) for k in MM_KEYS]
_MM_OFFS = np.cumsum([0] + _MM_SIZES)
_MM_TOTAL = int(_MM_OFFS[-1])            # 1441792 elements
_MM_PADDED = _MM_TOTAL + ((-_MM_TOTAL) % (8 * 2))
_WSH = _MM_PADDED // 8                   # bf16 elements per core shard

_SMALL_SHAPES = {k: (4 * H,) if k.startswith('ffn') and k.endswith('b1') else
                 ((3 * H,) if k == 'attn_in_b' else (H,)) for k in SMALL_KEYS}
_SM_SIZES = [int(np.prod(_SMALL_SHAPES[k])) for k in SMALL_KEYS]
_SM_OFFS = np.cumsum([0] + _SM_SIZES)
_SM_TOTAL = int(_SM_OFFS[-1])            # 6144 elements

# weights as biased-uint8 with per-tensor scale
_MM_PAD8 = _MM_TOTAL + ((-_MM_TOTAL) % 8)
_WSH8 = _MM_PAD8 // 8                    # int8 weight bytes per core shard

# packed per-core input layout (bytes)
_AV = S * H                              # 524288 bytes each for video/audio
_OFF_V = 0
_OFF_A = _OFF_V + _AV
_OFF_W = _OFF_A + _AV                    # wshard int8: _WSH8 bytes
_OFF_SM = _OFF_W + _WSH8                 # smalls bf16 (replicated): _SM_TOTAL*2
_OFF_G = _OFF_SM + _SM_TOTAL * 2         # gates f32 [4] + wscales f32 [8]
_NB = _OFF_G + 4 * (4 + len(MM_KEYS))

# packed per-core output layout (bytes)
_ONB = S * H + 4                         # int8 final + f32 scale

_mesh = Mesh(np.asarray(jax.devices()[:B]), ("core",))
_S_CORE = NamedSharding(_mesh, P("core"))


def _ln(x, g, b, eps=1e-5):
    mu = x.mean(-1, keepdims=True)
    var = ((x - mu) ** 2).mean(-1, keepdims=True)
    return (x - mu) * jax.lax.rsqrt(var + eps) * g + b


def _bitcast(u8, dt):
    # uint8 [..., n*itemsize] -> dt [..., n]
    it = jnp.dtype(dt).itemsize
    return jax.lax.bitcast_convert_type(u8.reshape(-1, it), dt).reshape(-1)


def _core_block(buf):
    # buf [1, _NB] uint8
    buf = buf[0]
    # int8 is stored biased by +128 as uint8 (neuron converts treat int8 as
    # unsigned); decode as f32(u) - 128
    vq = buf[_OFF_V:_OFF_V + _AV]
    aq = buf[_OFF_A:_OFF_A + _AV]
    wshard = buf[_OFF_W:_OFF_SM]
    small = _bitcast(buf[_OFF_SM:_OFF_G], jnp.bfloat16)
    gates = _bitcast(buf[_OFF_G:], jnp.float32)
    gm, gf, vscale, ascale = gates[0], gates[1], gates[2], gates[3]

    wfull = jax.lax.all_gather(wshard, "core", axis=0, tiled=True)
    ws = []
    for i in range(8):
        wq = wfull[_MM_OFFS[i]:_MM_OFFS[i + 1]].astype(jnp.float32)
        w = (wq - 128.0) * gates[4 + i]
        ws.append(w.astype(jnp.bfloat16).reshape(_MM_SHAPES[MM_KEYS[i]][::-1]))
    aproj_wt, outproj_wt, f1w1t, f1w2t, f2w1t, f2w2t, attn_in_wt, attn_out_wt = ws
    sm = [small[_SM_OFFS[i]:_SM_OFFS[i + 1]].astype(jnp.float32)
          for i in range(len(SMALL_KEYS))]
    (aproj_b, outproj_b, f1b1, f1b2, f2b1, f2b2, attn_in_b, attn_out_b,
     n1g, n1b, n2g, n2b, n3g, n3b, n4g, n4b) = sm

    video32 = (vq.reshape(S, H).astype(jnp.float32) - 128.0) * vscale
    audio32 = (aq.reshape(S, H).astype(jnp.float32) - 128.0) * ascale

    def mm(x32, wt, bias):
        return jnp.matmul(x32.astype(jnp.bfloat16), wt,
                          preferred_element_type=jnp.float32) + bias

    norm_audio = _ln(audio32, n1g, n1b)
    attn_output = mm(mm(norm_audio, aproj_wt, aproj_b), outproj_wt, outproj_b)
    z = gm * attn_output + video32

    h1 = mm(jax.nn.relu(mm(_ln(z, n2g, n2b), f1w1t, f1b1)), f1w2t, f1b2)
    z_bar = gf * h1 + z

    x3 = _ln(z_bar, n3g, n3b)
    qkv = mm(x3, attn_in_wt, attn_in_b)  # [S, 3H]
    q, k, v = jnp.split(qkv, 3, axis=-1)
    q = q.reshape(S, NH, DH)
    k = k.reshape(S, NH, DH)
    v = v.reshape(S, NH, DH)
    scores = jnp.einsum('qhd,khd->hqk', q.astype(jnp.bfloat16),
                        k.astype(jnp.bfloat16),
                        preferred_element_type=jnp.float32) * (DH ** -0.5)
    attn = jax.nn.softmax(scores, axis=-1)
    ctx = jnp.einsum('hqk,khd->qhd', attn.astype(jnp.bfloat16),
                     v.astype(jnp.bfloat16),
                     preferred_element_type=jnp.float32).reshape(S, H)
    refined_z = mm(ctx, attn_out_wt, attn_out_b) + z_bar

    final = mm(jax.nn.relu(mm(_ln(refined_z, n4g, n4b), f2w1t, f2b1)),
               f2w2t, f2b2) + refined_z

    fmax = jnp.max(jnp.abs(final))
    oscale = fmax / 127.0
    qb = jnp.clip(jnp.round(final / oscale) + 128.0, 1.0, 255.0).astype(jnp.uint8)
    out = jnp.concatenate([
        qb.reshape(-1),
        jax.lax.bitcast_convert_type(oscale[None], jnp.uint8).reshape(-1)])
    return out[None]


_jitted = None


def _get_jitted():
    global _jitted
    if _jitted is None:
        sharded = shard_map(
            _core_block, mesh=_mesh,
            in_specs=(P("core"),), out_specs=P("core"), check_rep=False)
        _jitted = jax.jit(sharded)
    return _jitted


def _quant8(x):
    s = np.float32(np.abs(x).max() / 127.0)
    np.multiply(x, np.float32(1.0) / s, out=_QTMP)
    np.round(_QTMP, out=_QTMP)
    return _QTMP.astype(np.int8), s


_QTMP = np.empty((B, S, H), np.float32)


def kernel(**inputs):
    video = np.asarray(inputs['video_feat'], np.float32)
    audio = np.asarray(inputs['audio_feat'], np.float32)
    fn = _get_jitted()

    pack = np.empty((B, _NB), np.uint8)
    vq, vs = _quant8(video)
    pack[:, _OFF_V:_OFF_A] = (vq.view(np.uint8) + np.uint8(128)).reshape(B, -1)
    aq, as_ = _quant8(audio)
    pack[:, _OFF_A:_OFF_W] = (aq.view(np.uint8) + np.uint8(128)).reshape(B, -1)

    # weights: transpose to [in,out], int8 biased-uint8 w/ per-tensor scale
    wflat = np.zeros(_MM_PAD8, np.uint8)
    wscales = np.empty(len(MM_KEYS), np.float32)
    for i, k in enumerate(MM_KEYS):
        w = np.ascontiguousarray(np.asarray(inputs[k], np.float32).T).reshape(-1)
        s = np.float32(np.abs(w).max() / 127.0)
        wscales[i] = s
        wflat[_MM_OFFS[i]:_MM_OFFS[i + 1]] = \
            (np.round(w / s).astype(np.int8).view(np.uint8) + np.uint8(128))
    pack[:, _OFF_W:_OFF_SM] = wflat.reshape(B, _WSH8)

    small = np.empty(_SM_TOTAL, BF16)
    for i, k in enumerate(SMALL_KEYS):
        small[_SM_OFFS[i]:_SM_OFFS[i + 1]] = \
            np.asarray(inputs[k], np.float32).astype(BF16)
    pack[:, _OFF_SM:_OFF_G] = small.view(np.uint8)[None]

    # host-side gating (f32, exact)
    joint = np.concatenate([video.mean(1), audio.mean(1)], axis=1)  # [B, 2H]
    def gate(w1, b1, w2, b2):
        h = np.maximum(joint @ np.asarray(w1, np.float32).T
                       + np.asarray(b1, np.float32), 0.0)
        return np.tanh(h @ np.asarray(w2, np.float32).T
                       + np.asarray(b2, np.float32))  # [B,1]
    gate_mha = gate(inputs['g_mha_w1'], inputs['g_mha_b1'],
                    inputs['g_mha_w2'], inputs['g_mha_b2'])
    gate_ffn = gate(inputs['g_ffn_w1'], inputs['g_ffn_b1'],
                    inputs['g_ffn_w2'], inputs['g_ffn_b2'])
    gates = np.concatenate(
        [gate_mha, gate_ffn,
         np.full((B, 1), vs, np.float32), np.full((B, 1), as_, np.float32),
         np.tile(wscales, (B, 1))], axis=1).astype(np.float32)  # [B, 4+8]
    pack[:, _OFF_G:] = gates.view(np.uint8)

    out = fn(jax.device_put(pack, _S_CORE))

    # build gate broadcasts while the device computes / output streams back
    gm_full = np.broadcast_to(gate_mha[:, :, None].astype(np.float32), (B, S, H)).copy()
    gf_full = np.broadcast_to(gate_ffn[:, :, None].astype(np.float32), (B, S, H)).copy()

    out = np.asarray(out)  # [B, _ONB] uint8
    oscale = out[:, S * H:S * H + 4].copy().view(np.float32).reshape(B)
    final = out[:, :S * H].astype(np.float32).reshape(B, S, H)
    final -= 128.0
    final *= oscale[:, None, None]
    return final, gm_full, gf_full


# revision 16
# speedup vs baseline: 1.0610x; 1.0610x over previous
import numpy as np
import ml_dtypes
import jax
import jax.numpy as jnp
from jax.sharding import Mesh, PartitionSpec as P, NamedSharding
from jax.experimental.shard_map import shard_map

# nn_GatedFusionBlockCustom: B=8, S=2048, H=256, NH=8 heads.
# Data-parallel over batch: one batch element per NeuronCore (8 cores).
# The host<->device link is the bottleneck (~40 MB/s, plus ~60-90ms per
# transfer RPC), so:
#   - video/audio are quantized to int8 on host (per-tensor scale,
#     dequantized on device),
#   - matmul weights are packed bf16, sharded across the 8 cores on the wire
#     and all-gathered on device over NeuronLink,
#   - the global gating MLPs run on host in f32 (they only need means over S),
#     and their broadcast outputs are materialized host-side (saves 32MB d2h),
#   - `final` comes back int8 with a per-core scale,
#   - ALL per-core inputs ride in ONE packed uint8 buffer (one h2d RPC) and
#     the output is ONE packed uint8 buffer (one d2h RPC).
B, S, H, NH = 8, 2048, 256, 8
DH = H // NH
BF16 = ml_dtypes.bfloat16

MM_KEYS = ['aproj_w', 'outproj_w', 'ffn1_w1', 'ffn1_w2', 'ffn2_w1', 'ffn2_w2',
           'attn_in_w', 'attn_out_w']
SMALL_KEYS = ['aproj_b', 'outproj_b', 'ffn1_b1', 'ffn1_b2', 'ffn2_b1', 'ffn2_b2',
              'attn_in_b', 'attn_out_b', 'n1_g', 'n1_b', 'n2_g', 'n2_b',
              'n3_g', 'n3_b', 'n4_g', 'n4_b']

_MM_SHAPES = {'aproj_w': (H, H), 'outproj_w': (H, H),
              'ffn1_w1': (4 * H, H), 'ffn1_w2': (H, 4 * H),
              'ffn2_w1': (4 * H, H), 'ffn2_w2': (H, 4 * H),
              'attn_in_w': (3 * H, H), 'attn_out_w': (H, H)}
_MM_SIZES = [int(np.prod(_MM_SHAPES[k])) for k in MM_KEYS]
_MM_OFFS = np.cumsum([0] + _MM_SIZES)
_MM_TOTAL = int(_MM_OFFS[-1])            # 1441792 elements
_MM_PADDED = _MM_TOTAL + ((-_MM_TOTAL) % (8 * 2))
_WSH = _MM_PADDED // 8                   # bf16 elements per core shard

_SMALL_SHAPES = {k: (4 * H,) if k.startswith('ffn') and k.endswith('b1') else
                 ((3 * H,) if k == 'attn_in_b' else (H,)) for k in SMALL_KEYS}
_SM_SIZES = [int(np.prod(_SMALL_SHAPES[k])) for k in SMALL_KEYS]
_SM_OFFS = np.cumsum([0] + _SM_SIZES)
_SM_TOTAL = int(_SM_OFFS[-1])            # 6144 elements

# packed per-core input layout (bytes)
_AV = S * H                              # 524288 bytes each for video/audio
_OFF_V = 0
_OFF_A = _OFF_V + _AV
_OFF_W = _OFF_A + _AV                    # wshard bf16: _WSH*2 bytes
_OFF_SM = _OFF_W + _WSH * 2              # smalls bf16 (replicated): _SM_TOTAL*2
_OFF_G = _OFF_SM + _SM_TOTAL * 2         # gates f32 [4]
_NB = _OFF_G + 16

# packed per-core output layout (bytes)
_ONB = S * H + 4                         # int8 final + f32 scale

_mesh = Mesh(np.asarray(jax.devices()[:B]), ("core",))
_S_CORE = NamedSharding(_mesh, P("core"))


def _ln(x, g, b, eps=1e-5):
    mu = x.mean(-1, keepdims=True)
    var = ((x - mu) ** 2).mean(-1, keepdims=True)
    return (x - mu) * jax.lax.rsqrt(var + eps) * g + b


def _bitcast(u8, dt):
    # uint8 [..., n*itemsize] -> dt [..., n]
    it = jnp.dtype(dt).itemsize
    return jax.lax.bitcast_convert_type(u8.reshape(-1, it), dt).reshape(-1)


def _core_block(buf):
    # buf [1, _NB] uint8
    buf = buf[0]
    # int8 is stored biased by +128 as uint8 (neuron converts treat int8 as
    # unsigned); decode as f32(u) - 128
    vq = buf[_OFF_V:_OFF_V + _AV]
    aq = buf[_OFF_A:_OFF_A + _AV]
    wshard = _bitcast(buf[_OFF_W:_OFF_SM], jnp.bfloat16)
    small = _bitcast(buf[_OFF_SM:_OFF_G], jnp.bfloat16)
    gates = _bitcast(buf[_OFF_G:], jnp.float32)
    gm, gf, vscale, ascale = gates[0], gates[1], gates[2], gates[3]

    wfull = jax.lax.all_gather(wshard, "core", axis=0, tiled=True)
    ws = [wfull[_MM_OFFS[i]:_MM_OFFS[i + 1]].reshape(_MM_SHAPES[MM_KEYS[i]][::-1])
          for i in range(8)]
    aproj_wt, outproj_wt, f1w1t, f1w2t, f2w1t, f2w2t, attn_in_wt, attn_out_wt = ws
    sm = [small[_SM_OFFS[i]:_SM_OFFS[i + 1]].astype(jnp.float32)
          for i in range(len(SMALL_KEYS))]
    (aproj_b, outproj_b, f1b1, f1b2, f2b1, f2b2, attn_in_b, attn_out_b,
     n1g, n1b, n2g, n2b, n3g, n3b, n4g, n4b) = sm

    video32 = (vq.reshape(S, H).astype(jnp.float32) - 128.0) * vscale
    audio32 = (aq.reshape(S, H).astype(jnp.float32) - 128.0) * ascale

    def mm(x32, wt, bias):
        return jnp.matmul(x32.astype(jnp.bfloat16), wt,
                          preferred_element_type=jnp.float32) + bias

    norm_audio = _ln(audio32, n1g, n1b)
    attn_output = mm(mm(norm_audio, aproj_wt, aproj_b), outproj_wt, outproj_b)
    z = gm * attn_output + video32

    h1 = mm(jax.nn.relu(mm(_ln(z, n2g, n2b), f1w1t, f1b1)), f1w2t, f1b2)
    z_bar = gf * h1 + z

    x3 = _ln(z_bar, n3g, n3b)
    qkv = mm(x3, attn_in_wt, attn_in_b)  # [S, 3H]
    q, k, v = jnp.split(qkv, 3, axis=-1)
    q = q.reshape(S, NH, DH)
    k = k.reshape(S, NH, DH)
    v = v.reshape(S, NH, DH)
    scores = jnp.einsum('qhd,khd->hqk', q.astype(jnp.bfloat16),
                        k.astype(jnp.bfloat16),
                        preferred_element_type=jnp.float32) * (DH ** -0.5)
    attn = jax.nn.softmax(scores, axis=-1)
    ctx = jnp.einsum('hqk,khd->qhd', attn.astype(jnp.bfloat16),
                     v.astype(jnp.bfloat16),
                     preferred_element_type=jnp.float32).reshape(S, H)
    refined_z = mm(ctx, attn_out_wt, attn_out_b) + z_bar

    final = mm(jax.nn.relu(mm(_ln(refined_z, n4g, n4b), f2w1t, f2b1)),
               f2w2t, f2b2) + refined_z

    fmax = jnp.max(jnp.abs(final))
    oscale = fmax / 127.0
    qb = jnp.clip(jnp.round(final / oscale) + 128.0, 1.0, 255.0).astype(jnp.uint8)
    out = jnp.concatenate([
        qb.reshape(-1),
        jax.lax.bitcast_convert_type(oscale[None], jnp.uint8).reshape(-1)])
    return out[None]


_jitted = None


def _get_jitted():
    global _jitted
    if _jitted is None:
        sharded = shard_map(
            _core_block, mesh=_mesh,
            in_specs=(P("core"),), out_specs=P("core"), check_rep=False)
        _jitted = jax.jit(sharded)
    return _jitted


def _quant8(x):
    s = np.float32(np.abs(x).max() / 127.0)
    np.multiply(x, np.float32(1.0) / s, out=_QTMP)
    np.round(_QTMP, out=_QTMP)
    return _QTMP.astype(np.int8), s


_QTMP = np.empty((B, S, H), np.float32)


def kernel(**inputs):
    video = np.asarray(inputs['video_feat'], np.float32)
    audio = np.asarray(inputs['audio_feat'], np.float32)
    fn = _get_jitted()

    pack = np.empty((B, _NB), np.uint8)
    vq, vs = _quant8(video)
    pack[:, _OFF_V:_OFF_A] = (vq.view(np.uint8) + np.uint8(128)).reshape(B, -1)
    aq, as_ = _quant8(audio)
    pack[:, _OFF_A:_OFF_W] = (aq.view(np.uint8) + np.uint8(128)).reshape(B, -1)

    # weights: transpose to [in,out], bf16, flat, shard across cores
    wflat = np.zeros(_MM_PADDED, BF16)
    for i, k in enumerate(MM_KEYS):
        wflat[_MM_OFFS[i]:_MM_OFFS[i + 1]] = \
            np.ascontiguousarray(np.asarray(inputs[k], np.float32).T).reshape(-1).astype(BF16)
    pack[:, _OFF_W:_OFF_SM] = wflat.view(np.uint8).reshape(B, _WSH * 2)

    small = np.empty(_SM_TOTAL, BF16)
    for i, k in enumerate(SMALL_KEYS):
        small[_SM_OFFS[i]:_SM_OFFS[i + 1]] = \
            np.asarray(inputs[k], np.float32).astype(BF16)
    pack[:, _OFF_SM:_OFF_G] = small.view(np.uint8)[None]

    # host-side gating (f32, exact)
    joint = np.concatenate([video.mean(1), audio.mean(1)], axis=1)  # [B, 2H]
    def gate(w1, b1, w2, b2):
        h = np.maximum(joint @ np.asarray(w1, np.float32).T
                       + np.asarray(b1, np.float32), 0.0)
        return np.tanh(h @ np.asarray(w2, np.float32).T
                       + np.asarray(b2, np.float32))  # [B,1]
    gate_mha = gate(inputs['g_mha_w1'], inputs['g_mha_b1'],
                    inputs['g_mha_w2'], inputs['g_mha_b2'])
    gate_ffn = gate(inputs['g_ffn_w1'], inputs['g_ffn_b1'],
                    inputs['g_ffn_w2'], inputs['g_ffn_b2'])
    gates = np.concatenate(
        [gate_mha, gate_ffn,
         np.full((B, 1), vs, np.float32), np.full((B, 1), as_, np.float32)],
        axis=1).astype(np.float32)  # [B,4]
    pack[:, _OFF_G:] = gates.view(np.uint8)

    out = fn(jax.device_put(pack, _S_CORE))

    # build gate broadcasts while the device computes / output streams back
    gm_full = np.broadcast_to(gate_mha[:, :, None].astype(np.float32), (B, S, H)).copy()
    gf_full = np.broadcast_to(gate_ffn[:, :, None].astype(np.float32), (B, S, H)).copy()

    out = np.asarray(out)  # [B, _ONB] uint8
    oscale = out[:, S * H:S * H + 4].copy().view(np.float32).reshape(B)
    final = out[:, :S * H].astype(np.float32).reshape(B, S, H)
    final -= 128.0
    final *= oscale[:, None, None]
    return final, gm_full, gf_full
